# revision 63
# baseline (speedup 1.0000x reference)
"""Multi-head causal attention (B=4, T=2048, DM=1024, H=16, DK=DV=64) on 8 TRN2
NeuronCores.

Sharding: tensor-parallel over heads — core c owns heads {2c, 2c+1}. Each core
projects Q^T/K^T/V for its 2 heads, runs causal attention per batch in the
S^T = K Q^T layout, row-shards W_o, and the host sums the 8 partials.

v2 schedule: the whole kernel is ONE interleaved stream built to keep the PE
back-to-back (TRN2's HAM clock-gate halves the PE clock whenever the engine
micro-stalls, so density IS speed):
  - attention runs in PW=512 q-pairs; both heads' scores live in one fused
    PSUM tile [128, 1024] (2 banks), double-buffered => QK(j+1) never waits
    on the exp drain of S(j);
  - one scalar ACTIVATE per j exps both heads (PSUM->SBUF bf16);
  - PV is emitted one j behind QK so the PE never waits head-of-line on exp;
  - softmax denominators ride as a ones-column in V (PV out row 64); the
    per-pair normalize (d-row copy -> K=1 broadcast matmul -> reciprocal ->
    scale) is deferred into the next pair's second iteration;
  - projection t-blocks are emitted just-in-time before the pair that first
    needs them (so proj matmuls fill the scalar catch-up windows), and W_o
    matmuls for the previous batch are drip-fed as fillers inside the j loop;
  - PSUM: tag "s" 2 x [128,1024] (S tiles / proj / Wo / dbc rotate here)
    + tag "acc" 4 x [65->128, 512], pairs alternate slot pairs => no
    write-after-read coupling between consecutive pairs. 4+4 = 8 banks.
"""

import sys

for _p in ("/opt/trn_rl_repo",):
    if _p not in sys.path:
        sys.path.insert(0, _p)

import numpy as np

# ---- problem constants (hardcoded per harness contract) ----
B, T, DM = 4, 2048, 1024
H, DK = 16, 64
NCORES = 8
HL = 2                      # heads per core
SD = HL * DK                # 128: partition width of per-core head-stacked tiles
BT = B * T

# tiling
TB = 512                    # projection t-block (moving N) == PW
PW = 512                    # attention q pair width
ND = DM // 128              # contraction k-tiles for projections
NT = T // 128               # 128-wide s-tiles per batch
NPAIR = T // PW             # 4 pairs per batch
TBB = T // TB               # 4 projection t-blocks per batch
SPT = PW // 128             # 4 s-tiles per pair width
SCALE = DK ** -0.5

_CACHE = {}


def _build():
    import concourse.bass as bass
    import concourse.tile as tile
    from concourse import bacc, mybir

    f32 = mybir.dt.float32
    bf16 = mybir.dt.bfloat16
    ts = bass.ts

    nc = bacc.Bacc("TRN2", target_bir_lowering=False, debug=False,
                   num_devices=NCORES)

    xT = nc.dram_tensor("xT", [DM, BT], bf16, kind="ExternalInput").ap()
    wq2 = nc.dram_tensor("wq2", [DM, SD], bf16, kind="ExternalInput").ap()
    wk2 = nc.dram_tensor("wk2", [DM, SD], bf16, kind="ExternalInput").ap()
    wv2 = nc.dram_tensor("wv2", [DM, SD], bf16, kind="ExternalInput").ap()
    wo_my = nc.dram_tensor("wo_my", [SD, DM], bf16, kind="ExternalInput").ap()
    outp = nc.dram_tensor("out_part", [BT, DM], bf16, kind="ExternalOutput").ap()

    def dma_in(out, in_):  # SWDGE casts on the fly; HWDGE needs equal dtypes
        if out.dtype != in_.dtype:
            return nc.gpsimd.dma_start(out=out, in_=in_)
        return nc.sync.dma_start(out=out, in_=in_)

    # additive causal mask in S^T layout: -1e30 where q < s (strict lower)
    tri = ((1.0 - np.triu(np.ones((128, 128)))) * -1e30).astype(np.float32)
    # broadcast row lives at partition 64 so its base partition matches the
    # accumulator denominator row it multiplies (matmul requires equal bases)
    ones2 = np.zeros((65, 64), np.float32)
    ones2[64, :] = 1.0
    vones = np.ones((128, NT, 1), np.float32)
    tri_h = nc.inline_tensor(tri, name="tri_const")
    vones_h = nc.inline_tensor(vones, name="vones_const")
    ones2_h = nc.inline_tensor(ones2, name="ones2_const")

    with tile.TileContext(nc) as tc:
        with (
            tc.tile_pool(name="singles", bufs=1) as singles,
            tc.tile_pool(name="stream", bufs=3) as stream,
            tc.tile_pool(name="seq", bufs=2) as seq,
            tc.tile_pool(name="att", bufs=4) as att,
            tc.tile_pool(name="small", bufs=4) as small,
            tc.tile_pool(name="ps", bufs=2, space="PSUM") as ps,
            tc.tile_pool(name="pacc", bufs=2, space="PSUM") as pacc,
            tc.tile_pool(name="pw1", bufs=2, space="PSUM") as pw1,
        ):
            xT_r = xT.rearrange("(a p) t -> p a t", p=128)
            # the batch-0 first x-tile heads the DMA queue: it gates the
            # very first matmul together with wq
            xts0 = stream.tile([128, ND, TB], bf16, tag="xts", name="xts0")
            nc.sync.dma_start(out=xts0, in_=xT_r[:, :, 0:TB])
            w_sb = {}
            for name, src in (("q", wq2), ("k", wk2), ("v", wv2)):
                w_sb[name] = singles.tile([128, ND, SD], bf16, tag=f"w{name}",
                                          name=f"w{name}_sb")
                nc.sync.dma_start(
                    out=w_sb[name], in_=src.rearrange("(a p) m -> p a m", p=128)
                )

            tri_sb = singles.tile([128, 128], f32, tag="tri")
            ones2_sb = singles.tile([65, 64], bf16, tag="ones2")
            wo_sb = singles.tile([128, DM], bf16, tag="wo")
            nc.sync.dma_start(out=tri_sb, in_=tri_h.ap())
            dma_in(ones2_sb, ones2_h.ap())
            dma_in(wo_sb, wo_my)

            # per-batch sequence tiles (bufs=2: batch b read while b+1 written)
            cur = {}

            def emit_proj_block(b, i):
                """Project q/k/v of batch b, t-block i (t in [512i, 512i+512)).
                q/k weight-stationary -> q^T/k^T [dk2, t] (PSUM, DVE drain);
                v x-stationary -> V [t, v0|1|v1|1] directly."""
                if i == 0:
                    qt = seq.tile([128, T], bf16, tag="qt")
                    kt = seq.tile([128, T], bf16, tag="kt")
                    vsb = seq.tile([128, NT * 130], bf16, tag="vsb")
                    onorm = seq.tile([128, T], bf16, tag="onorm")
                    cur[b] = (qt, kt, vsb, onorm)
                    vsb3 = vsb.rearrange("p (n c) -> p n c", c=130)
                    dma_in(vsb3[:, :, 64:65], vones_h.ap())
                    dma_in(vsb3[:, :, 129:130], vones_h.ap())
                qt, kt, vsb, _ = cur[b]
                vsb3 = vsb.rearrange("p (n c) -> p n c", c=130)
                if b == 0 and i == 0:
                    xts = xts0
                else:
                    xts = stream.tile([128, ND, TB], bf16, tag="xts")
                    nc.sync.dma_start(
                        out=xts,
                        in_=xT_r[:, :, b * T + i * TB: b * T + (i + 1) * TB],
                    )
                for name, dst in (("q", qt), ("k", kt)):
                    # w1 tag: the "s" slots at a boundary still hold the
                    # pair's last two un-exp'd S tiles, so a pj there waits
                    # on the exp tail; the w1 slots (po chain) are drained
                    pj = pw1.tile([128, TB], f32, tag="w1", name="pj")
                    for a in range(ND):
                        nc.tensor.matmul(
                            pj, w_sb[name][:, a, :], xts[:, a, :],
                            start=(a == 0), stop=(a == ND - 1),
                        )
                    nc.vector.tensor_copy(dst[:, ts(i, TB)], pj)

                for half in range(TB // 128):
                    j = (TB // 128) * i + half   # 128-wide t-tile index
                    pv = ps.tile([128, 128], f32, tag="s", name="pv")
                    for a in range(ND):
                        nc.tensor.matmul(
                            pv, xts[:, a, ts(half, 128)], w_sb["v"][:, a, :],
                            start=(a == 0), stop=(a == ND - 1),
                        )
                    # scatter [t,128] -> [t, v0|_, v1|_] as two contiguous
                    # per-head copies; the h1 copy rides the ACT engine, which
                    # has drained its exp tail and idles during the boundary —
                    # halves the DVE backlog the next pair's first QKs wait on
                    nc.vector.tensor_copy(vsb3[:, j, 0:64], pv[:, 0:64])
                    nc.scalar.copy(vsb3[:, j, 65:129], pv[:, 64:128])

            # ---- Wo fillers: one unit = one 512-wide chunk of one t-tile;
            # drains split across scalar and vector so neither paces ----
            wo_queue = []
            osb_hold = {}

            def make_wo_unit(b, tc_i, cc, tail=False):
                def unit():
                    onorm = cur[b][3]
                    if cc == 0:
                        osb_hold[0] = stream.tile([128, DM], bf16, tag="osb",
                                                  name="osb")
                    osb = osb_hold[0]
                    po = pw1.tile([128, 512], f32, tag="w1", name="po")
                    nc.tensor.matmul(
                        po, onorm[:, ts(tc_i, 128)], wo_sb[:, ts(cc, 512)],
                        start=True, stop=True,
                    )
                    # DVE only: a scalar-engine drain would insert ~650ns
                    # into the exp stream that paces the attention phase
                    if tail and cc % 2 == 1:
                        # tail only: the exp stream is finished, the ACT
                        # engine idles — split drains to halve the tail pace
                        nc.scalar.copy(osb[:, ts(cc, 512)], po)
                    else:
                        # DVE otherwise: a scalar drain would queue behind
                        # the exp stream and stall the next po via w1
                        nc.vector.tensor_copy(osb[:, ts(cc, 512)], po)
                    if cc == 1:
                        r0 = b * T + tc_i * 128
                        nc.sync.dma_start(out=outp[r0:r0 + 128, :], in_=osb)
                return unit

            def queue_wo(b, tail=False):
                for tc_i in range(NT):
                    for cc in range(2):
                        wo_queue.append(make_wo_unit(b, tc_i, cc, tail))

            def pop_wo(n=1):
                for _ in range(n):
                    if wo_queue:
                        wo_queue.pop(0)()

            pending_norm = [None]

            def make_norm(b, p, acc):
                """Deferred per-pair normalize: d-row -> SBUF (partition 64),
                K=1 broadcast matmul into a dbc tile in the "s" rotation,
                fast reciprocal, then scale O' straight out of PSUM."""
                def norm():
                    onorm = cur[b][3]
                    rx = []
                    for h in (0, 1):
                        dsb = small.tile([65, PW], bf16, tag="dsb", name="dsb")
                        nc.vector.tensor_copy(dsb[64:65, :], acc[h][64:65, :])
                        dbc = pw1.tile([64, PW], f32, tag="w1", name="dbc")
                        nc.tensor.matmul(
                            dbc, ones2_sb[64:65, :], dsb[64:65, :],
                            start=True, stop=True,
                        )
                        rcp = small.tile([64, PW], f32, tag="rcp", name="rcp")
                        nc.vector.reciprocal_approx_fast(out=rcp, in_=dbc)
                        rx.append(rcp)
                    nc.vector.tensor_mul(
                        onorm[0:64, ts(p, PW)], acc[0][0:64, :], rx[0],
                    )
                    # h1 rows are produced at base partition 0 (DVE lanes are
                    # partition-locked) and DMA'd across to partitions 64-127
                    on1 = small.tile([64, PW], bf16, tag="on1", name="on1")
                    nc.vector.tensor_mul(on1, acc[1][0:64, :], rx[1])
                    nc.sync.dma_start(
                        out=onorm[64:128, ts(p, PW)], in_=on1,
                    )
                return norm

            def emit_pair(b, p, queue_batch):
                """Attention for batch b, q-pair p (q in [512p, 512p+512)).
                queue_batch: batch whose Wo units become fillers once the
                previous pair's normalize has been flushed (None = none)."""
                qt, kt, vsb, _ = cur[b]
                vsb3 = vsb.rearrange("p (n c) -> p n c", c=130)
                nj = SPT * (p + 1)
                acc = [pacc.tile([65, PW], f32, tag="acc", name="acc")
                       for _ in (0, 1)]
                pend = []            # PV skew 2: up to two (j, E, c0) pending

                def emit_pv(j, E, c0):
                    for h in (0, 1):
                        nc.tensor.matmul(
                            acc[h][:, c0:PW],
                            vsb3[:, j, h * 65:h * 65 + 65],
                            E[:, h * PW + c0:(h + 1) * PW],
                            start=(j == 0), stop=(j == nj - 1),
                            skip_group_check=True,
                        )

                for j in range(nj):
                    j_rel = j - SPT * p
                    c0 = max(0, 128 * j_rel)
                    S = ps.tile([128, 2 * PW], f32, tag="s", name="S")
                    for h in (0, 1):
                        nc.tensor.matmul(
                            S[:, h * PW + c0:(h + 1) * PW],
                            kt[h * 64:(h + 1) * 64, ts(j, 128)],
                            qt[h * 64:(h + 1) * 64, p * PW + c0:p * PW + PW],
                            start=True, stop=True,
                        )
                    if j_rel >= 0:               # diagonal: mask on PSUM
                        for h in (0, 1):
                            nc.vector.tensor_add(
                                S[:, h * PW + c0:h * PW + c0 + 128],
                                S[:, h * PW + c0:h * PW + c0 + 128],
                                tri_sb,
                            )
                    E = att.tile([128, 2 * PW], bf16, tag="expt", name="E")
                    if c0 == 0:
                        # one linear activation covers both heads
                        nc.scalar.activation(
                            out=E, in_=S,
                            func=mybir.ActivationFunctionType.Exp,
                            scale=SCALE,
                        )
                    else:
                        # two linear activations: a strided 3D ap on the ACT
                        # engine measures ~2x slower than linear
                        for h in (0, 1):
                            nc.scalar.activation(
                                out=E[:, h * PW + c0:(h + 1) * PW],
                                in_=S[:, h * PW + c0:(h + 1) * PW],
                                func=mybir.ActivationFunctionType.Exp,
                                scale=SCALE,
                            )
                    if j == 1 and pending_norm[0] is not None:
                        # previous pair's normalize: deferred here so the DVE
                        # d-row copies drain during this pair's first QKs
                        pending_norm[0]()
                        pending_norm[0] = None
                        if queue_batch is not None:
                            queue_wo(queue_batch)
                    elif j > 1:
                        # Wo fillers (pw1 chain) keep the PE ahead of the
                        # exp stream; they are independent of attention
                        pop_wo(1 if c0 == 0 else 2)
                    pend.append((j, E, c0))
                    if len(pend) > 2:
                        emit_pv(*pend.pop(0))
                for it in pend:
                    emit_pv(*it)
                pending_norm[0] = make_norm(b, p, acc)

            # ---- the interleaved stream ----
            for g in range(B * NPAIR):
                b, p = divmod(g, NPAIR)
                pop_wo(3)
                emit_proj_block(b, p)
                emit_pair(b, p,
                          queue_batch=(b - 1) if (p == 0 and b > 0) else None)
            pending_norm[0]()          # last pair's normalize
            queue_wo(B - 1, tail=True)
            pop_wo(len(wo_queue))      # tail: drain all remaining Wo units

    nc.compile()
    return nc


def _get_nc():
    if "nc" not in _CACHE:
        _CACHE["nc"] = _build()
    return _CACHE["nc"]


def make_in_maps(x, Wq, Wk, Wv, Wo, bo):
    import ml_dtypes
    hdt = ml_dtypes.bfloat16
    x2d = np.ascontiguousarray(x.reshape(BT, DM), dtype=np.float32)
    xT = np.ascontiguousarray(x2d.T).astype(hdt)
    maps = []
    for c in range(NCORES):
        h0, h1 = HL * c, HL * c + 1
        maps.append({
            "xT": xT,
            "wq2": np.ascontiguousarray(
                np.concatenate([Wq[h0], Wq[h1]], 1)).astype(hdt),
            "wk2": np.ascontiguousarray(
                np.concatenate([Wk[h0], Wk[h1]], 1)).astype(hdt),
            "wv2": np.ascontiguousarray(
                np.concatenate([Wv[h0], Wv[h1]], 1)).astype(hdt),
            "wo_my": np.ascontiguousarray(
                Wo[SD * c: SD * (c + 1)]).astype(hdt),
        })
    return maps


def run(x, Wq, Wk, Wv, Wo, bo, trace=False, **spmd_kwargs):
    from concourse.bass_utils import run_bass_kernel_spmd

    nc = _get_nc()
    maps = make_in_maps(x, Wq, Wk, Wv, Wo, bo)
    res = run_bass_kernel_spmd(
        nc, maps, core_ids=list(range(NCORES)), trace=trace, **spmd_kwargs
    )
    total = np.zeros((BT, DM), np.float32)
    for r in res.results:
        total += r["out_part"].astype(np.float32)
    total += np.asarray(bo, dtype=np.float32)[None, :]
    return total.reshape(B, T, DM), res


def kernel(x, Wq, Wk, Wv, Wo, bo):
    out, _ = run(x, Wq, Wk, Wv, Wo, bo)
    return out
